# revision 12
# baseline (speedup 1.0000x reference)
"""AFPM (adaptive per-patch modulation) kernel for 8 TRN2 NeuronCores.

Reference computation (B=8, C=64, H=W=512, K=8, HID=64):
  - d[l]: normalized distance of each 8x8 patch center from image center
  - pk[l, kk] / pb[l]: tiny MLPs of d (host-precomputable, data-independent)
  - feats[b,c,l] = sum_kk patches[b,c,kk,l] * pk[l,kk] + pb[l]
  - feats2 = conv_w @ feats + conv_b           (1x1 conv over channels)
  - out patches = patches * feats2[:, :, None, :]

Sharding (v2): core i handles patch-rows i*8..i*8+7 for ALL 8 images.
Tile (t, v) = patch-row t (of 8), image-pair v (of 4); partitions =
(u, c) = image-in-pair x channel; free = (dy, pw, dx) = 4096.

Engine split per tile (all x traffic in bf16; 2 MiB DMA per tile):
  DMA  in  : xb [128,4096] bf16, rings alternate sync/tensor by v
  DVE  mul : PROD = xb * PKREP[t]                  (TT 2x, 2048 cyc)
  Pool L1/2: dy-halving adds 8->4->2 in bf16       (offloads DVE)
  DVE  red : f[p,pw] = reduce_{dy2,dx}(T2) f32     (1024 cyc 1x)
  PE   g   : g = blockdiag(conv_w.T).T @ f  (+)  w2.T @ [pb;1]  (PSUM acc)
  ACT gexp : bf16(g) expanded over dx (dense 512 run)
  DVE  out : OUT = xb * bcast(gexp) into PROD buf  (TT 2x, 2048 cyc)
  DMA  out : rings alternate scalar/vector by v

pkr is never shipped replicated: the [8,4096] table DMAs once; each
row is broadcast to 128 partitions on-device by a PE ones-matmul into
PSUM chunks + ACT copy to bf16 SBUF, pipelined one row ahead.
"""

import math
import sys

import numpy as np

for _p in ("/opt/trn_rl_repo",):
    if _p not in sys.path:
        sys.path.insert(0, _p)

import concourse.bass as bass
import concourse.tile as tile
from concourse import bacc, mybir
from concourse.bass_utils import run_bass_kernel_spmd

B, C, H, W, K, HID = 8, 64, 512, 512, 8, 64
NH, NW = H // K, W // K          # 64, 64
L = NH * NW                      # 4096
NR = 8                           # patch-rows per core
NV = 4                           # image-pairs per row
NT = NR * NV                     # 32 tiles per core
FD = K * W                       # 4096 free dim per tile
F32 = mybir.dt.float32
BF16 = mybir.dt.bfloat16

_ERF = np.frompyfunc(math.erf, 1, 1)


def _gelu(x):
    x = np.asarray(x, np.float64)
    return 0.5 * x * (1.0 + _ERF(x / math.sqrt(2.0)).astype(np.float64))


def _host_tables(w1k, b1k, w2k, b2k, w1b, b1b, w2b, b2b, conv_w, conv_b):
    """pk/pb via the tiny MLPs; packed as PKR [NH, FD] plus fold consts."""
    cy = cx = H / 2.0
    max_d = math.sqrt(cy * cy + cx * cx)
    py = np.arange(NH, dtype=np.float64) * K + K / 2.0
    px = np.arange(NW, dtype=np.float64) * K + K / 2.0
    d = np.sqrt((py - cy)[:, None] ** 2 + (px - cx)[None, :] ** 2) / max_d
    d = d.reshape(L, 1)

    pk = _gelu(d @ w1k.astype(np.float64) + b1k) @ w2k.astype(np.float64) + b2k
    pb = (_gelu(d @ w1b.astype(np.float64) + b1b) @ w2b.astype(np.float64) + b2b)[:, 0]

    import ml_dtypes

    # PKR[ph, dy*W + pw*K + dx] = pk[ph*NW + pw, dy*K + dx]  (bf16 on device)
    pkr = (
        pk.reshape(NH, NW, K, K).transpose(0, 2, 1, 3).reshape(NH, FD)
    ).astype(ml_dtypes.bfloat16)

    # g = bd.T @ f  +  w2.T @ [pb_row; 1]   (rank-2 fold of pb and conv_b)
    cw1 = conv_w.astype(np.float64).sum(axis=1)
    w2 = np.stack([np.tile(cw1, 2), np.tile(conv_b.astype(np.float64), 2)]).astype(
        np.float32
    )  # [2, 128]

    bd = np.zeros((128, 128), np.float32)
    bd[0:C, 0:C] = conv_w.T
    bd[C:128, C:128] = conv_w.T
    return pkr, pb, w2, bd, pk


def build_program():
    nc = bacc.Bacc("TRN2", target_bir_lowering=False, debug=False, num_devices=8)
    x_d = nc.dram_tensor("x", [NV, 128, NR * K, W], BF16, kind="ExternalInput")
    pkrtab_d = nc.dram_tensor("pkrtab", [NR, FD], BF16, kind="ExternalInput")
    pbx_d = nc.dram_tensor("pbx", [2, NR * NW], F32, kind="ExternalInput")
    w2_d = nc.dram_tensor("w2", [2, 128], F32, kind="ExternalInput")
    bd_d = nc.dram_tensor("bd", [128, 128], F32, kind="ExternalInput")
    wsel_d = nc.dram_tensor("wsel", [NR, NR * 128], BF16, kind="ExternalInput")
    out_d = nc.dram_tensor("out", [NV, 128, NR * K, W], BF16, kind="ExternalOutput")

    # [t, v, p=(u c), dy, w] views of the DRAM image slices
    xr = x_d.ap().rearrange("v p (t dy) w -> t v p dy w", dy=K)
    outr = out_d.ap().rearrange("v p (t dy) w -> t v p dy w", dy=K)

    NCHUNK = FD // 512  # 8 PE-broadcast chunks per pkr row

    with tile.TileContext(nc) as tc:
        with (
            tc.tile_pool(name="const", bufs=1) as constp,
            tc.tile_pool(name="xbp", bufs=6) as xbp,
            tc.tile_pool(name="prodp", bufs=4) as prodp,
            tc.tile_pool(name="t1p", bufs=3) as t1p,
            tc.tile_pool(name="t2p", bufs=3) as t2p,
            tc.tile_pool(name="pkrepp", bufs=2) as pkrepp,
            tc.tile_pool(name="smallp", bufs=4) as smallp,
            tc.tile_pool(name="gpsum", bufs=3, space="PSUM") as gpsum,
            tc.tile_pool(name="rpsum", bufs=2, space="PSUM") as rpsum,
        ):
            pkrtab = constp.tile([NR, FD], BF16)
            nc.sync.dma_start(pkrtab[:], pkrtab_d[:])
            pbx = constp.tile([2, NR * NW], F32)
            nc.sync.dma_start(pbx[:], pbx_d[:])
            w2t = constp.tile([2, 128], F32)
            nc.sync.dma_start(w2t[:], w2_d[:])
            bdt = constp.tile([128, 128], F32)
            nc.sync.dma_start(bdt[:], bd_d[:])
            wselt = constp.tile([NR, NR * 128], BF16)
            nc.sync.dma_start(wselt[:], wsel_d[:])

            pkreps = [None] * NR

            def emit_repl_chunk(t, j):
                """PE-broadcast chunk j of pkr row t into pkreps[t] (bf16)."""
                ch = rpsum.tile([128, 512], F32)
                nc.tensor.matmul(
                    ch[:],
                    wselt[:, t * 128 : (t + 1) * 128],
                    pkrtab[:, j * 512 : (j + 1) * 512],
                    start=True,
                    stop=True,
                )
                nc.scalar.copy(pkreps[t][:, j * 512 : (j + 1) * 512], ch[:])

            # prologue: replicate row 0 fully
            pkreps[0] = pkrepp.tile([128, FD], BF16, tag="pkrep", name="pkrep0")
            for j in range(NCHUNK):
                emit_repl_chunk(0, j)

            for t in range(NR):
                for v in range(NV):
                    xb = xbp.tile([128, FD], BF16)
                    (nc.sync if v % 2 == 0 else nc.scalar).dma_start(
                        xb.rearrange("p (dy w) -> p dy w", dy=K), xr[t, v]
                    )

                    prod = prodp.tile([128, FD], BF16)
                    nc.vector.tensor_mul(prod[:], xb[:], pkreps[t][:])

                    # dy-halving adds on Pool (bf16 partial sums)
                    with nc.allow_low_precision("pairwise bf16 tree adds"):
                        t1 = t1p.tile([128, FD // 2], BF16)
                        nc.gpsimd.tensor_tensor(
                            t1[:],
                            prod[:, 0 : FD // 2],
                            prod[:, FD // 2 : FD],
                            op=mybir.AluOpType.add,
                        )
                        t2 = t2p.tile([128, FD // 4], BF16)
                        nc.gpsimd.tensor_tensor(
                            t2[:],
                            t1[:, 0 : FD // 4],
                            t1[:, FD // 4 : FD // 2],
                            op=mybir.AluOpType.add,
                        )

                    f = smallp.tile([128, NW], F32)
                    nc.vector.tensor_reduce(
                        f[:],
                        t2.rearrange("p (dy2 pw dx) -> p pw dy2 dx", dy2=2, pw=NW),
                        axis=mybir.AxisListType.XY,
                        op=mybir.AluOpType.add,
                    )

                    g = gpsum.tile([128, NW], F32)
                    nc.tensor.matmul(g[:], bdt[:], f[:], start=True, stop=False)
                    nc.tensor.matmul(
                        g[:],
                        w2t[:],
                        pbx[:, t * NW : (t + 1) * NW],
                        start=False,
                        stop=True,
                    )

                    # cast g to bf16 expanded over dx (dense 512-elem inner run)
                    gexp = smallp.tile([128, NW * K], BF16, tag="gexp")
                    ge3 = gexp.rearrange("p (pw dx) -> p pw dx", dx=K)
                    gs3 = g.rearrange("p (pw a) -> p pw a", a=1)
                    ge3b, gs3b = bass.broadcast_tensor_aps(ge3, gs3)
                    nc.scalar.copy(ge3b, gs3b)

                    # OUT = xb * bcast(gexp) over dy, bf16, into PROD's buffer
                    o3 = prod.rearrange("p (dy q) -> p dy q", dy=K)
                    x3 = xb.rearrange("p (dy q) -> p dy q", dy=K)
                    g3 = gexp.rearrange("p (a q) -> p a q", a=1)
                    x3b, g3b = bass.broadcast_tensor_aps(x3, g3)
                    nc.vector.tensor_tensor(o3, x3b, g3b, op=mybir.AluOpType.mult)

                    (nc.scalar if v % 2 == 0 else nc.sync).dma_start(
                        outr[t, v], prod.rearrange("p (dy w) -> p dy w", dy=K)
                    )

                    # pipeline next row's pkr broadcast: 2 chunks per tile
                    if t + 1 < NR:
                        if v == 0:
                            pkreps[t + 1] = pkrepp.tile(
                                [128, FD], BF16, tag="pkrep", name=f"pkrep{t + 1}"
                            )
                        for j in (2 * v, 2 * v + 1):
                            emit_repl_chunk(t + 1, j)

    nc.compile()
    return nc


_PROGRAM = None
LAST_RESULT = None


def make_in_maps(x, pkr, pb, w2, bd):
    import ml_dtypes

    # wsel[c, t*128 + p] = 1 if c == t: lhsT column block t selects pkr row t
    wsel = np.zeros((NR, NR, 128), np.float32)
    for t in range(NR):
        wsel[t, t, :] = 1.0
    wsel = wsel.reshape(NR, NR * 128).astype(ml_dtypes.bfloat16)
    in_maps = []
    for i in range(8):
        r0 = i * NR
        x_core = (
            np.ascontiguousarray(x[:, :, r0 * K : (r0 + NR) * K, :])
            .astype(ml_dtypes.bfloat16)
            .reshape(NV, 128, NR * K, W)
        )
        pkrtab = np.ascontiguousarray(pkr[r0 : r0 + NR])
        pbx = np.empty((2, NR * NW), np.float32)
        pbx[0] = pb[r0 * NW : (r0 + NR) * NW]
        pbx[1] = 1.0
        in_maps.append(
            {
                "x": x_core,
                "pkrtab": pkrtab,
                "pbx": pbx,
                "w2": w2,
                "bd": bd,
                "wsel": wsel,
            }
        )
    return in_maps


def kernel(**inputs):
    global _PROGRAM, LAST_RESULT
    x = np.ascontiguousarray(np.asarray(inputs["x"], dtype=np.float32))
    pkr, pb, w2, bd, pk = _host_tables(
        *[
            np.asarray(inputs[k], dtype=np.float32)
            for k in (
                "w1k", "b1k", "w2k", "b2k",
                "w1b", "b1b", "w2b", "b2b",
                "conv_w", "conv_b",
            )
        ]
    )
    if _PROGRAM is None:
        _PROGRAM = build_program()
    nc = _PROGRAM

    in_maps = make_in_maps(x, pkr, pb, w2, bd)

    conv_w = np.asarray(inputs["conv_w"], np.float64)
    conv_b = np.asarray(inputs["conv_b"], np.float64)

    def _spot_check(out):
        """Verify a sample of patches against the exact host formula;
        catches the rare silent device corruption (bf16 path ~0.4%/elem)."""
        rng = np.random.default_rng(1234)
        worst = 0.0
        for _ in range(32):
            b = int(rng.integers(B))
            ph = int(rng.integers(NH))
            pw = int(rng.integers(NW))
            l = ph * NW + pw
            patch = x[b, :, ph * K : (ph + 1) * K, pw * K : (pw + 1) * K]
            patch = patch.reshape(C, K * K).astype(np.float64)
            feats = patch @ pk[l] + pb[l]
            g = conv_w @ feats + conv_b
            exp = patch * g[:, None]
            got = out[b, :, ph * K : (ph + 1) * K, pw * K : (pw + 1) * K]
            got = got.reshape(C, K * K).astype(np.float64)
            denom = np.linalg.norm(exp) + 1e-30
            worst = max(worst, float(np.linalg.norm(got - exp) / denom))
        return worst

    res = None
    for attempt in range(4):
        try:
            res = run_bass_kernel_spmd(nc, in_maps, list(range(8)))
        except Exception:
            if attempt == 3:
                raise
            continue
        out = np.empty((B, C, H, W), np.float32)
        for i in range(8):
            r0 = i * NR
            out[:, :, r0 * K : (r0 + NR) * K, :] = (
                res.results[i]["out"].astype(np.float32).reshape(B, C, NR * K, W)
            )
        err = _spot_check(out)
        if err < 0.05:
            break
        if attempt == 3:
            raise RuntimeError(f"device output failed spot check ({err:.3f})")
    LAST_RESULT = res
    return out


# revision 13
# speedup vs baseline: 1.4716x; 1.4716x over previous
"""AFPM (adaptive per-patch modulation) kernel for 8 TRN2 NeuronCores.

Reference computation (B=8, C=64, H=W=512, K=8, HID=64):
  - d[l]: normalized distance of each 8x8 patch center from image center
  - pk[l, kk] / pb[l]: tiny MLPs of d (host-precomputable, data-independent)
  - feats[b,c,l] = sum_kk patches[b,c,kk,l] * pk[l,kk] + pb[l]
  - feats2 = conv_w @ feats + conv_b           (1x1 conv over channels)
  - out patches = patches * feats2[:, :, None, :]

Sharding (v2): core i handles patch-rows i*8..i*8+7 for ALL 8 images.
Tile (t, v) = patch-row t (of 8), image-pair v (of 4); partitions =
(u, c) = image-in-pair x channel; free = (dy, pw, dx) = 4096.

Engine split per tile (all x traffic in bf16; 2 MiB DMA per tile):
  DMA  in  : xb [128,4096] bf16, rings alternate sync/tensor by v
  DVE  mul : PROD = xb * PKREP[t]                  (TT 2x, 2048 cyc)
  Pool L1/2: dy-halving adds 8->4->2 in bf16       (offloads DVE)
  DVE  red : f[p,pw] = reduce_{dy2,dx}(T2) f32     (1024 cyc 1x)
  PE   g   : g = blockdiag(conv_w.T).T @ f  (+)  w2.T @ [pb;1]  (PSUM acc)
  ACT gexp : bf16(g) expanded over dx (dense 512 run)
  DVE  out : OUT = xb * bcast(gexp) into PROD buf  (TT 2x, 2048 cyc)
  DMA  out : rings alternate scalar/vector by v

pkr is never shipped replicated: the [8,4096] table DMAs once; each
row is broadcast to 128 partitions on-device by a PE ones-matmul into
PSUM chunks + ACT copy to bf16 SBUF, pipelined one row ahead.
"""

import math
import sys

import numpy as np

for _p in ("/opt/trn_rl_repo",):
    if _p not in sys.path:
        sys.path.insert(0, _p)

import concourse.bass as bass
import concourse.tile as tile
from concourse import bacc, mybir
from concourse.bass_utils import run_bass_kernel_spmd

B, C, H, W, K, HID = 8, 64, 512, 512, 8, 64
NH, NW = H // K, W // K          # 64, 64
L = NH * NW                      # 4096
NR = 8                           # patch-rows per core
NV = 4                           # image-pairs per row
NT = NR * NV                     # 32 tiles per core
FD = K * W                       # 4096 free dim per tile
F32 = mybir.dt.float32
BF16 = mybir.dt.bfloat16

_ERF = np.frompyfunc(math.erf, 1, 1)


def _gelu(x):
    x = np.asarray(x, np.float64)
    return 0.5 * x * (1.0 + _ERF(x / math.sqrt(2.0)).astype(np.float64))


def _host_tables(w1k, b1k, w2k, b2k, w1b, b1b, w2b, b2b, conv_w, conv_b):
    """pk/pb via the tiny MLPs; packed as PKR [NH, FD] plus fold consts."""
    cy = cx = H / 2.0
    max_d = math.sqrt(cy * cy + cx * cx)
    py = np.arange(NH, dtype=np.float64) * K + K / 2.0
    px = np.arange(NW, dtype=np.float64) * K + K / 2.0
    d = np.sqrt((py - cy)[:, None] ** 2 + (px - cx)[None, :] ** 2) / max_d
    d = d.reshape(L, 1)

    pk = _gelu(d @ w1k.astype(np.float64) + b1k) @ w2k.astype(np.float64) + b2k
    pb = (_gelu(d @ w1b.astype(np.float64) + b1b) @ w2b.astype(np.float64) + b2b)[:, 0]

    import ml_dtypes

    # PKR[ph, dy*W + pw*K + dx] = pk[ph*NW + pw, dy*K + dx]  (bf16 on device)
    pkr = (
        pk.reshape(NH, NW, K, K).transpose(0, 2, 1, 3).reshape(NH, FD)
    ).astype(ml_dtypes.bfloat16)

    # g = bd.T @ f  +  w2.T @ [pb_row; 1]   (rank-2 fold of pb and conv_b)
    cw1 = conv_w.astype(np.float64).sum(axis=1)
    w2 = np.stack([np.tile(cw1, 2), np.tile(conv_b.astype(np.float64), 2)]).astype(
        np.float32
    )  # [2, 128]

    bd = np.zeros((128, 128), np.float32)
    bd[0:C, 0:C] = conv_w.T
    bd[C:128, C:128] = conv_w.T
    return pkr, pb, w2, bd, pk


def build_program():
    nc = bacc.Bacc("TRN2", target_bir_lowering=False, debug=False, num_devices=8)
    x_d = nc.dram_tensor("x", [NV, 128, NR * K, W], BF16, kind="ExternalInput")
    pkrtab_d = nc.dram_tensor("pkrtab", [NR, FD], BF16, kind="ExternalInput")
    pbx_d = nc.dram_tensor("pbx", [2, NR * NW], F32, kind="ExternalInput")
    w2_d = nc.dram_tensor("w2", [2, 128], F32, kind="ExternalInput")
    bd_d = nc.dram_tensor("bd", [128, 128], F32, kind="ExternalInput")
    wsel_d = nc.dram_tensor("wsel", [NR, NR * 128], BF16, kind="ExternalInput")
    out_d = nc.dram_tensor("out", [NV, 128, NR * K, W], BF16, kind="ExternalOutput")

    # [t, v, p=(u c), dy, w] views of the DRAM image slices
    xr = x_d.ap().rearrange("v p (t dy) w -> t v p dy w", dy=K)
    outr = out_d.ap().rearrange("v p (t dy) w -> t v p dy w", dy=K)

    NCHUNK = FD // 512  # 8 PE-broadcast chunks per pkr row

    with tile.TileContext(nc) as tc:
        with (
            tc.tile_pool(name="const", bufs=1) as constp,
            tc.tile_pool(name="xbp", bufs=6) as xbp,
            tc.tile_pool(name="prodp", bufs=4) as prodp,
            tc.tile_pool(name="t1p", bufs=3) as t1p,
            tc.tile_pool(name="t2p", bufs=3) as t2p,
            tc.tile_pool(name="pkrepp", bufs=2) as pkrepp,
            tc.tile_pool(name="smallp", bufs=4) as smallp,
            tc.tile_pool(name="gpsum", bufs=3, space="PSUM") as gpsum,
            tc.tile_pool(name="rpsum", bufs=2, space="PSUM") as rpsum,
        ):
            pkrtab = constp.tile([NR, FD], BF16)
            nc.sync.dma_start(pkrtab[:], pkrtab_d[:])
            pbx = constp.tile([2, NR * NW], F32)
            nc.sync.dma_start(pbx[:], pbx_d[:])
            w2t = constp.tile([2, 128], F32)
            nc.sync.dma_start(w2t[:], w2_d[:])
            bdt = constp.tile([128, 128], F32)
            nc.sync.dma_start(bdt[:], bd_d[:])
            wselt = constp.tile([NR, NR * 128], BF16)
            nc.sync.dma_start(wselt[:], wsel_d[:])

            pkreps = [None] * NR

            def emit_repl_chunk(t, j):
                """PE-broadcast chunk j of pkr row t into pkreps[t] (bf16)."""
                ch = rpsum.tile([128, 512], F32)
                nc.tensor.matmul(
                    ch[:],
                    wselt[:, t * 128 : (t + 1) * 128],
                    pkrtab[:, j * 512 : (j + 1) * 512],
                    start=True,
                    stop=True,
                )
                nc.scalar.copy(pkreps[t][:, j * 512 : (j + 1) * 512], ch[:])

            # prologue: replicate row 0 fully
            pkreps[0] = pkrepp.tile([128, FD], BF16, tag="pkrep", name="pkrep0")
            for j in range(NCHUNK):
                emit_repl_chunk(0, j)

            for t in range(NR):
                for v in range(NV):
                    xb = xbp.tile([128, FD], BF16)
                    (nc.sync if v % 2 == 0 else nc.scalar).dma_start(
                        xb.rearrange("p (dy w) -> p dy w", dy=K), xr[t, v]
                    )

                    prod = prodp.tile([128, FD], BF16)
                    nc.vector.tensor_mul(prod[:], xb[:], pkreps[t][:])

                    # dy-halving adds on DVE (bf16 partial sums, 2x mode)
                    with nc.allow_low_precision("pairwise bf16 tree adds"):
                        t1 = t1p.tile([128, FD // 2], BF16)
                        nc.vector.tensor_tensor(
                            t1[:],
                            prod[:, 0 : FD // 2],
                            prod[:, FD // 2 : FD],
                            op=mybir.AluOpType.add,
                        )
                        t2 = t2p.tile([128, FD // 4], BF16)
                        nc.vector.tensor_tensor(
                            t2[:],
                            t1[:, 0 : FD // 4],
                            t1[:, FD // 4 : FD // 2],
                            op=mybir.AluOpType.add,
                        )

                    f = smallp.tile([128, NW], F32)
                    nc.vector.tensor_reduce(
                        f[:],
                        t2.rearrange("p (dy2 pw dx) -> p pw dy2 dx", dy2=2, pw=NW),
                        axis=mybir.AxisListType.XY,
                        op=mybir.AluOpType.add,
                    )

                    g = gpsum.tile([128, NW], F32)
                    nc.tensor.matmul(g[:], bdt[:], f[:], start=True, stop=False)
                    nc.tensor.matmul(
                        g[:],
                        w2t[:],
                        pbx[:, t * NW : (t + 1) * NW],
                        start=False,
                        stop=True,
                    )

                    # cast g to bf16 expanded over dx (dense 512-elem inner run)
                    gexp = smallp.tile([128, NW * K], BF16, tag="gexp")
                    ge3 = gexp.rearrange("p (pw dx) -> p pw dx", dx=K)
                    gs3 = g.rearrange("p (pw a) -> p pw a", a=1)
                    ge3b, gs3b = bass.broadcast_tensor_aps(ge3, gs3)
                    nc.scalar.copy(ge3b, gs3b)

                    # OUT = xb * bcast(gexp) over dy, bf16, into PROD's buffer
                    o3 = prod.rearrange("p (dy q) -> p dy q", dy=K)
                    x3 = xb.rearrange("p (dy q) -> p dy q", dy=K)
                    g3 = gexp.rearrange("p (a q) -> p a q", a=1)
                    x3b, g3b = bass.broadcast_tensor_aps(x3, g3)
                    nc.vector.tensor_tensor(o3, x3b, g3b, op=mybir.AluOpType.mult)

                    (nc.scalar if v % 2 == 0 else nc.sync).dma_start(
                        outr[t, v], prod.rearrange("p (dy w) -> p dy w", dy=K)
                    )

                    # pipeline next row's pkr broadcast: 2 chunks per tile
                    if t + 1 < NR:
                        if v == 0:
                            pkreps[t + 1] = pkrepp.tile(
                                [128, FD], BF16, tag="pkrep", name=f"pkrep{t + 1}"
                            )
                        for j in (2 * v, 2 * v + 1):
                            emit_repl_chunk(t + 1, j)

    nc.compile()
    return nc


_PROGRAM = None
LAST_RESULT = None


def make_in_maps(x, pkr, pb, w2, bd):
    import ml_dtypes

    # wsel[c, t*128 + p] = 1 if c == t: lhsT column block t selects pkr row t
    wsel = np.zeros((NR, NR, 128), np.float32)
    for t in range(NR):
        wsel[t, t, :] = 1.0
    wsel = wsel.reshape(NR, NR * 128).astype(ml_dtypes.bfloat16)
    in_maps = []
    for i in range(8):
        r0 = i * NR
        x_core = (
            np.ascontiguousarray(x[:, :, r0 * K : (r0 + NR) * K, :])
            .astype(ml_dtypes.bfloat16)
            .reshape(NV, 128, NR * K, W)
        )
        pkrtab = np.ascontiguousarray(pkr[r0 : r0 + NR])
        pbx = np.empty((2, NR * NW), np.float32)
        pbx[0] = pb[r0 * NW : (r0 + NR) * NW]
        pbx[1] = 1.0
        in_maps.append(
            {
                "x": x_core,
                "pkrtab": pkrtab,
                "pbx": pbx,
                "w2": w2,
                "bd": bd,
                "wsel": wsel,
            }
        )
    return in_maps


def kernel(**inputs):
    global _PROGRAM, LAST_RESULT
    x = np.ascontiguousarray(np.asarray(inputs["x"], dtype=np.float32))
    pkr, pb, w2, bd, pk = _host_tables(
        *[
            np.asarray(inputs[k], dtype=np.float32)
            for k in (
                "w1k", "b1k", "w2k", "b2k",
                "w1b", "b1b", "w2b", "b2b",
                "conv_w", "conv_b",
            )
        ]
    )
    if _PROGRAM is None:
        _PROGRAM = build_program()
    nc = _PROGRAM

    in_maps = make_in_maps(x, pkr, pb, w2, bd)

    conv_w = np.asarray(inputs["conv_w"], np.float64)
    conv_b = np.asarray(inputs["conv_b"], np.float64)

    def _spot_check(out):
        """Verify a sample of patches against the exact host formula;
        catches the rare silent device corruption (bf16 path ~0.4%/elem)."""
        rng = np.random.default_rng(1234)
        worst = 0.0
        for _ in range(32):
            b = int(rng.integers(B))
            ph = int(rng.integers(NH))
            pw = int(rng.integers(NW))
            l = ph * NW + pw
            patch = x[b, :, ph * K : (ph + 1) * K, pw * K : (pw + 1) * K]
            patch = patch.reshape(C, K * K).astype(np.float64)
            feats = patch @ pk[l] + pb[l]
            g = conv_w @ feats + conv_b
            exp = patch * g[:, None]
            got = out[b, :, ph * K : (ph + 1) * K, pw * K : (pw + 1) * K]
            got = got.reshape(C, K * K).astype(np.float64)
            denom = np.linalg.norm(exp) + 1e-30
            worst = max(worst, float(np.linalg.norm(got - exp) / denom))
        return worst

    res = None
    for attempt in range(4):
        try:
            res = run_bass_kernel_spmd(nc, in_maps, list(range(8)))
        except Exception:
            if attempt == 3:
                raise
            continue
        out = np.empty((B, C, H, W), np.float32)
        for i in range(8):
            r0 = i * NR
            out[:, :, r0 * K : (r0 + NR) * K, :] = (
                res.results[i]["out"].astype(np.float32).reshape(B, C, NR * K, W)
            )
        err = _spot_check(out)
        if err < 0.05:
            break
        if attempt == 3:
            raise RuntimeError(f"device output failed spot check ({err:.3f})")
    LAST_RESULT = res
    return out
